# revision 10
# baseline (speedup 1.0000x reference)
"""Trainium2 Bass kernel for nn_Attention_26920855011759.

Computes, per batch b (sharded 1 batch per NeuronCore, 8 cores):
    ci     = W_in @ context[b]^T                  [D, X]
    logits = output[b] @ ci                       [Y, X]
    logits = where(mask, -1e-6, logits)           (-1e-6 folded to 0.0: logits
                                                   are continuous ~N(0,32), the
                                                   1e-6 shift is ~1e-38 relative
                                                   after softmax)
    atten  = softmax(logits, axis=-1)             [Y, X]  (fp32 output)
    ctx    = atten @ context[b]                   [Y, E2]
    out    = tanh([ctx, output[b]] @ W_out^T + b) [Y, D]  (fp32 output)

Precision: the logit path (ci and logits matmuls) runs as 3-pass bf16
hi/lo split products (hi*hi + hi*lo + lo*hi, fp32 PSUM accumulation),
giving ~fp32-quality logits; softmax runs in fp32 on-chip; the ctx/out
matmuls run in fp16. Expected absmax error ~2e-4 (atten) / ~2e-3 (out).
"""

import sys

if "/opt/trn_rl_repo" not in sys.path:
    sys.path.insert(0, "/opt/trn_rl_repo")

import numpy as np
import ml_dtypes

import concourse.bass as bass
import concourse.bacc as bacc
import concourse.mybir as mybir
from concourse.tile import TileContext
from concourse.bass_utils import run_bass_kernel_spmd
from concourse.masks import make_identity

F32 = mybir.dt.float32
BF16 = mybir.dt.bfloat16
F16 = mybir.dt.float16

B, Y, X = 8, 2048, 2048
E2 = 1024  # encoder_hidden_size * 2 ("e" axis)
D = 1024   # decoder_hidden_size ("d" axis)
F = E2 + D # concat axis for the output projection
P = 128    # partitions
N_CORES = 8

YT = Y // P          # 16 y tiles
Y_MACRO = 512        # y macro-tile (M3/M4 free dim)
YM = Y // Y_MACRO    # 4 y macros
XS = 512             # x slice (M2 free dim / PSUM bank)
NXS = X // XS        # 4 x slices
KE = E2 // P         # 8 contraction tiles over e
KD = D // P          # 8 contraction tiles over d
KX = X // P          # 16 contraction tiles over x
ME = E2 // P         # 8 output-partition tiles over e (M3)
ND = D // XS         # 2 d slices (M4 free dim)

AluOp = mybir.AluOpType
ActFn = mybir.ActivationFunctionType


def build_nc():
    nc = bacc.Bacc()

    # --- DRAM parameters (per-core shard; names match host prep below) ---
    winT_hi = nc.declare_dram_parameter("winT_hi", [E2, D], BF16, isOutput=False)
    winT_lo = nc.declare_dram_parameter("winT_lo", [E2, D], BF16, isOutput=False)
    ctxT_hi = nc.declare_dram_parameter("ctxT_hi", [E2, X], BF16, isOutput=False)
    ctxT_lo = nc.declare_dram_parameter("ctxT_lo", [E2, X], BF16, isOutput=False)
    outT_hi = nc.declare_dram_parameter("outT_hi", [D, Y], BF16, isOutput=False)
    outT_lo = nc.declare_dram_parameter("outT_lo", [D, Y], BF16, isOutput=False)
    outT_f16 = nc.declare_dram_parameter("outT_f16", [D, Y], F16, isOutput=False)
    ctx_f16 = nc.declare_dram_parameter("ctx_f16", [X, E2], F16, isOutput=False)
    keep_f16 = nc.declare_dram_parameter("keep_f16", [Y, X], F16, isOutput=False)
    woutT_f16 = nc.declare_dram_parameter("woutT_f16", [F, D], F16, isOutput=False)
    bout_f16 = nc.declare_dram_parameter("bout_f16", [1, D], F16, isOutput=False)

    atten_out = nc.declare_dram_parameter("atten", [Y, X], F32, isOutput=True)
    out_out = nc.declare_dram_parameter("out", [Y, D], F32, isOutput=True)

    # tiled dram views: partition-major [p, tile, free]
    winT_hi_v = winT_hi.rearrange("(kt p) d -> p kt d", p=P)
    winT_lo_v = winT_lo.rearrange("(kt p) d -> p kt d", p=P)
    ctxT_hi_v = ctxT_hi.rearrange("(kt p) x -> p kt x", p=P)
    ctxT_lo_v = ctxT_lo.rearrange("(kt p) x -> p kt x", p=P)
    outT_hi_v = outT_hi.rearrange("(kt p) y -> p kt y", p=P)
    outT_lo_v = outT_lo.rearrange("(kt p) y -> p kt y", p=P)
    outT_f16_v = outT_f16.rearrange("(kt p) y -> p kt y", p=P)
    ctx_f16_v = ctx_f16.rearrange("(xt p) e -> p xt e", p=P)
    woutT_v = woutT_f16.rearrange("(kt p) d -> p kt d", p=P)

    with TileContext(nc) as tc:
        with (
            tc.tile_pool(name="res", bufs=1) as res,
            tc.tile_pool(name="ps_tr", bufs=2, space="PSUM") as ps_tr_pool,
            tc.tile_pool(name="ps_mm", bufs=2, space="PSUM") as ps_mm_pool,
            tc.tile_pool(name="ps_m34", bufs=2, space="PSUM") as ps_m34_pool,
        ):
            # ---------- resident tensors ----------
            ci_hi = res.tile([P, KD, X], BF16, tag="ci_hi")
            ci_lo = res.tile([P, KD, X], BF16, tag="ci_lo")
            ctx_t = res.tile([P, KX, E2], F16, tag="ctx_t")
            nc.sync.dma_start(out=ctx_t[:], in_=ctx_f16_v[:])
            wout_t = res.tile([P, F // P, D], F16, tag="wout_t")
            nc.sync.dma_start(out=wout_t[:], in_=woutT_v[:])
            ident = res.tile([P, P], F16, tag="ident")
            make_identity(nc, ident[:])
            ones_t = res.tile([1, P], F16, tag="ones_t")
            nc.vector.memset(ones_t[:], 1.0)
            bout_t = res.tile([1, D], F16, tag="bout_t")
            nc.sync.dma_start(out=bout_t[:], in_=bout_f16[:])

            # ---------- phase 1: ci = W_in @ context^T (split-3) ----------
            with (
                tc.tile_pool(name="ph1w", bufs=1) as ph1w,
                tc.tile_pool(name="ph1", bufs=2) as ph1,
            ):
                win_hi_t = ph1w.tile([P, KE, D], BF16, tag="win_hi")
                nc.sync.dma_start(out=win_hi_t[:], in_=winT_hi_v[:])
                win_lo_t = ph1w.tile([P, KE, D], BF16, tag="win_lo")
                nc.sync.dma_start(out=win_lo_t[:], in_=winT_lo_v[:])

                for nx in range(NXS):
                    cxh = ph1.tile([P, KE, XS], BF16, tag="cxh")
                    nc.sync.dma_start(
                        out=cxh[:], in_=ctxT_hi_v[:, :, nx * XS:(nx + 1) * XS]
                    )
                    cxl = ph1.tile([P, KE, XS], BF16, tag="cxl")
                    nc.sync.dma_start(
                        out=cxl[:], in_=ctxT_lo_v[:, :, nx * XS:(nx + 1) * XS]
                    )
                    for md in range(KD):
                        ps = ps_mm_pool.tile([P, XS], F32, tag="ps_logit")
                        for ke in range(KE):
                            wh = win_hi_t[:, ke, md * P:(md + 1) * P]
                            wl = win_lo_t[:, ke, md * P:(md + 1) * P]
                            nc.tensor.matmul(
                                ps[:], wh, cxh[:, ke, :],
                                start=(ke == 0), stop=False,
                            )
                            nc.tensor.matmul(ps[:], wh, cxl[:, ke, :],
                                             start=False, stop=False)
                            nc.tensor.matmul(ps[:], wl, cxh[:, ke, :],
                                             start=False, stop=(ke == KE - 1))
                        xsl = slice(nx * XS, (nx + 1) * XS)
                        nc.vector.tensor_copy(ci_hi[:, md, xsl], ps[:])
                        nc.vector.scalar_tensor_tensor(
                            out=ci_lo[:, md, xsl], in0=ps[:], scalar=1.0,
                            in1=ci_hi[:, md, xsl],
                            op0=AluOp.mult, op1=AluOp.subtract,
                        )

            # ---------- phase 2: per-y-tile pipeline ----------
            with (
                tc.tile_pool(name="stream", bufs=2) as stream,
                tc.tile_pool(name="stream1", bufs=1) as stream1,
                tc.tile_pool(name="stage", bufs=1) as stage,
                tc.tile_pool(name="e_pool", bufs=2) as e_pool,
            ):
                # per-macro tiles (bufs=1: PE program order serializes reuse)
                attenT = stage.tile([P, KX, Y_MACRO], F16, tag="attenT")
                ctxTout = stage.tile([P, ME, Y_MACRO], F16, tag="ctxTout")

                e_tiles = {}  # ytg -> e_f16 tile (lagged transpose reads)

                def emit_m2_softmax(ytg):
                    ysl = slice(ytg * P, (ytg + 1) * P)
                    oth = stream.tile([P, KD, P], BF16, tag="oth")
                    nc.sync.dma_start(out=oth[:], in_=outT_hi_v[:, :, ysl])
                    otl = stream.tile([P, KD, P], BF16, tag="otl")
                    nc.sync.dma_start(out=otl[:], in_=outT_lo_v[:, :, ysl])
                    keep_t = stream1.tile([P, X], F16, tag="keep_t")
                    nc.sync.dma_start(out=keep_t[:], in_=keep_f16[ysl, :])

                    L = stage.tile([P, X], F32, tag="L")
                    for nx in range(NXS):
                        ps = ps_mm_pool.tile([P, XS], F32, tag="ps_logit")
                        xsl = slice(nx * XS, (nx + 1) * XS)
                        for kd in range(KD):
                            lh = oth[:, kd, :]
                            ll = otl[:, kd, :]
                            nc.tensor.matmul(
                                ps[:], lh, ci_hi[:, kd, xsl],
                                start=(kd == 0), stop=False,
                            )
                            nc.tensor.matmul(ps[:], lh, ci_lo[:, kd, xsl],
                                             start=False, stop=False)
                            nc.tensor.matmul(ps[:], ll, ci_hi[:, kd, xsl],
                                             start=False, stop=(kd == KD - 1))
                        nc.vector.scalar_tensor_tensor(
                            out=L[:, xsl], in0=ps[:], scalar=1.0,
                            in1=keep_t[:, xsl],
                            op0=AluOp.mult, op1=AluOp.mult,
                        )

                    negmax = stage.tile([P, 1], F32, tag="negmax")
                    nc.vector.tensor_reduce(
                        out=negmax[:], in_=L[:], axis=mybir.AxisListType.X,
                        op=AluOp.max, negate=True,
                    )
                    e_t = stage.tile([P, X], F16, tag="e_t")
                    sums = stage.tile([P, 1], F32, tag="sums")
                    nc.scalar.activation(
                        out=e_t[:], in_=L[:], func=ActFn.Exp,
                        bias=negmax[:], scale=1.0, accum_out=sums[:],
                    )
                    recip = stage.tile([P, 1], F32, tag="recip")
                    nc.vector.reciprocal(recip[:], sums[:])
                    att_f16 = e_pool.tile([P, X], F16, tag="att_f16")
                    nc.vector.tensor_scalar_mul(att_f16[:], e_t[:], recip[:])
                    att_sb = stage.tile([P, X], F32, tag="att_sb")
                    nc.scalar.copy(att_sb[:], att_f16[:])
                    nc.sync.dma_start(out=atten_out[ysl, :], in_=att_sb[:])
                    e_tiles[ytg] = att_f16

                def emit_transposes(ytg):
                    e_t = e_tiles.pop(ytg)
                    yt = ytg % 4
                    for xt in range(KX):
                        ps_t = ps_tr_pool.tile([P, P], F16, tag="ps_t")
                        nc.tensor.transpose(
                            ps_t[:], e_t[:, xt * P:(xt + 1) * P], ident[:]
                        )
                        nc.scalar.copy(
                            attenT[:, xt, yt * P:(yt + 1) * P], ps_t[:]
                        )

                def emit_m3_m4(my):
                    # M3: ctxT[e', y_macro] = sum_x context[x, e'] * attenT[x, y]
                    for me in range(ME):
                        ps = ps_m34_pool.tile([P, Y_MACRO], F32, tag="ps_m34")
                        for kx in range(KX):
                            nc.tensor.matmul(
                                ps[:],
                                ctx_t[:, kx, me * P:(me + 1) * P],
                                attenT[:, kx, :],
                                start=(kx == 0), stop=(kx == KX - 1),
                            )
                        nc.scalar.copy(ctxTout[:, me, :], ps[:])

                    # M4 per y-tile of this macro
                    ot16 = stream1.tile([P, KD, Y_MACRO], F16, tag="ot16")
                    nc.sync.dma_start(
                        out=ot16[:],
                        in_=outT_f16_v[:, :, my * Y_MACRO:(my + 1) * Y_MACRO],
                    )
                    for yt in range(4):
                        ytg = my * 4 + yt
                        ysl_g = slice(ytg * P, (ytg + 1) * P)
                        ysl_m = slice(yt * P, (yt + 1) * P)
                        out_sb = stage.tile([P, D], F32, tag="out_sb")
                        for nd in range(ND):
                            ps = ps_m34_pool.tile([P, XS], F32, tag="ps_m34")
                            dsl = slice(nd * XS, (nd + 1) * XS)
                            for k in range(F // P):
                                if k < ME:
                                    lhsT = ctxTout[:, k, ysl_m]
                                else:
                                    lhsT = ot16[:, k - ME, ysl_m]
                                nc.tensor.matmul(
                                    ps[:], lhsT, wout_t[:, k, dsl],
                                    start=(k == 0), stop=False,
                                )
                            nc.tensor.matmul(
                                ps[:], ones_t[:, 0:P], bout_t[:, dsl],
                                start=False, stop=True,
                            )
                            nc.scalar.activation(
                                out=out_sb[:, dsl], in_=ps[:], func=ActFn.Tanh,
                            )
                        nc.sync.dma_start(out=out_out[ysl_g, :], in_=out_sb[:])

                # software-pipelined emission: transposes lag M2 by one tile
                for ytg in range(YT + 1):
                    if ytg < YT:
                        emit_m2_softmax(ytg)
                    if ytg >= 1:
                        emit_transposes(ytg - 1)
                        if (ytg - 1) % 4 == 3:
                            emit_m3_m4((ytg - 1) // 4)

    nc.compile()
    return nc


def _bf16_split(x32):
    hi = x32.astype(ml_dtypes.bfloat16)
    lo = (x32 - hi.astype(np.float32)).astype(ml_dtypes.bfloat16)
    return hi, lo


def host_prep(output, context, masks, W_in, W_out, b_out):
    """Build per-core input maps from the full inputs."""
    output = np.asarray(output, dtype=np.float32)
    context = np.asarray(context, dtype=np.float32)
    masks = np.asarray(masks)
    W_in = np.asarray(W_in, dtype=np.float32)
    W_out = np.asarray(W_out, dtype=np.float32)
    b_out = np.asarray(b_out, dtype=np.float32)

    winT = np.ascontiguousarray(W_in.T)           # [E2, D]
    winT_hi, winT_lo = _bf16_split(winT)
    woutT_f16 = np.ascontiguousarray(W_out.T).astype(np.float16)  # [F, D]
    bout_f16 = b_out.astype(np.float16).reshape(1, D)

    in_maps = []
    for b in range(B):
        ctxT = np.ascontiguousarray(context[b].T)  # [E2, X]
        ctxT_hi, ctxT_lo = _bf16_split(ctxT)
        outT = np.ascontiguousarray(output[b].T)   # [D, Y]
        outT_hi, outT_lo = _bf16_split(outT)
        in_maps.append({
            "winT_hi": winT_hi, "winT_lo": winT_lo,
            "ctxT_hi": ctxT_hi, "ctxT_lo": ctxT_lo,
            "outT_hi": outT_hi, "outT_lo": outT_lo,
            "outT_f16": outT.astype(np.float16),
            "ctx_f16": context[b].astype(np.float16),
            "keep_f16": (1 - masks[b]).astype(np.float16),
            "woutT_f16": woutT_f16,
            "bout_f16": bout_f16,
        })
    return in_maps


_NC_CACHE = {}


def _get_nc():
    if "nc" not in _NC_CACHE:
        _NC_CACHE["nc"] = build_nc()
    return _NC_CACHE["nc"]


def kernel(output, context, masks, W_in, W_out, b_out, _trace=False, _tmpdir=None):
    nc = _get_nc()
    in_maps = host_prep(output, context, masks, W_in, W_out, b_out)
    res = run_bass_kernel_spmd(
        nc, in_maps, list(range(N_CORES)), trace=_trace, tmpdir=_tmpdir
    )
    atten = np.stack([res.results[i]["atten"] for i in range(N_CORES)])
    out = np.stack([res.results[i]["out"] for i in range(N_CORES)])
    kernel.last_exec_time_ns = res.exec_time_ns
    kernel.last_trace = (
        res.instructions_and_trace[1] if res.instructions_and_trace else None
    )
    return (out, atten)


kernel.last_exec_time_ns = None
kernel.last_trace = None


# revision 11
# speedup vs baseline: 1.0089x; 1.0089x over previous
"""Trainium2 Bass kernel for nn_Attention_26920855011759.

Computes, per batch b (sharded 1 batch per NeuronCore, 8 cores):
    ci     = W_in @ context[b]^T                  [D, X]
    logits = output[b] @ ci                       [Y, X]
    logits = where(mask, -1e-6, logits)           (-1e-6 folded to 0.0: logits
                                                   are continuous ~N(0,32), the
                                                   1e-6 shift is ~1e-38 relative
                                                   after softmax)
    atten  = softmax(logits, axis=-1)             [Y, X]  (fp32 output)
    ctx    = atten @ context[b]                   [Y, E2]
    out    = tanh([ctx, output[b]] @ W_out^T + b) [Y, D]  (fp32 output)

Precision: the logit path (ci and logits matmuls) runs as 3-pass bf16
hi/lo split products (hi*hi + hi*lo + lo*hi, fp32 PSUM accumulation),
giving ~fp32-quality logits; softmax runs in fp32 on-chip; the ctx/out
matmuls run in fp16. Expected absmax error ~2e-4 (atten) / ~2e-3 (out).
"""

import sys

if "/opt/trn_rl_repo" not in sys.path:
    sys.path.insert(0, "/opt/trn_rl_repo")

import numpy as np
import ml_dtypes

import concourse.bass as bass
import concourse.bacc as bacc
import concourse.mybir as mybir
from concourse.tile import TileContext
from concourse.bass_utils import run_bass_kernel_spmd
from concourse.masks import make_identity

F32 = mybir.dt.float32
BF16 = mybir.dt.bfloat16
F16 = mybir.dt.float16

B, Y, X = 8, 2048, 2048
E2 = 1024  # encoder_hidden_size * 2 ("e" axis)
D = 1024   # decoder_hidden_size ("d" axis)
F = E2 + D # concat axis for the output projection
P = 128    # partitions
N_CORES = 8

YT = Y // P          # 16 y tiles
Y_MACRO = 512        # y macro-tile (M3/M4 free dim)
YM = Y // Y_MACRO    # 4 y macros
XS = 512             # x slice (M2 free dim / PSUM bank)
NXS = X // XS        # 4 x slices
KE = E2 // P         # 8 contraction tiles over e
KD = D // P          # 8 contraction tiles over d
KX = X // P          # 16 contraction tiles over x
ME = E2 // P         # 8 output-partition tiles over e (M3)
ND = D // XS         # 2 d slices (M4 free dim)

AluOp = mybir.AluOpType
ActFn = mybir.ActivationFunctionType


def build_nc():
    nc = bacc.Bacc()

    # --- DRAM parameters (per-core shard; names match host prep below) ---
    winT_hi = nc.declare_dram_parameter("winT_hi", [E2, D], BF16, isOutput=False)
    winT_lo = nc.declare_dram_parameter("winT_lo", [E2, D], BF16, isOutput=False)
    ctxT_hi = nc.declare_dram_parameter("ctxT_hi", [E2, X], BF16, isOutput=False)
    ctxT_lo = nc.declare_dram_parameter("ctxT_lo", [E2, X], BF16, isOutput=False)
    outT_hi = nc.declare_dram_parameter("outT_hi", [D, Y], BF16, isOutput=False)
    outT_lo = nc.declare_dram_parameter("outT_lo", [D, Y], BF16, isOutput=False)
    outT_f16 = nc.declare_dram_parameter("outT_f16", [D, Y], F16, isOutput=False)
    ctx_f16 = nc.declare_dram_parameter("ctx_f16", [X, E2], F16, isOutput=False)
    keep_f16 = nc.declare_dram_parameter("keep_f16", [Y, X], F16, isOutput=False)
    woutT_f16 = nc.declare_dram_parameter("woutT_f16", [F, D], F16, isOutput=False)
    bout_f16 = nc.declare_dram_parameter("bout_f16", [1, D], F16, isOutput=False)

    atten_out = nc.declare_dram_parameter("atten", [Y, X], F32, isOutput=True)
    out_out = nc.declare_dram_parameter("out", [Y, D], F32, isOutput=True)

    # tiled dram views: partition-major [p, tile, free]
    winT_hi_v = winT_hi.rearrange("(kt p) d -> p kt d", p=P)
    winT_lo_v = winT_lo.rearrange("(kt p) d -> p kt d", p=P)
    ctxT_hi_v = ctxT_hi.rearrange("(kt p) x -> p kt x", p=P)
    ctxT_lo_v = ctxT_lo.rearrange("(kt p) x -> p kt x", p=P)
    outT_hi_v = outT_hi.rearrange("(kt p) y -> p kt y", p=P)
    outT_lo_v = outT_lo.rearrange("(kt p) y -> p kt y", p=P)
    outT_f16_v = outT_f16.rearrange("(kt p) y -> p kt y", p=P)
    ctx_f16_v = ctx_f16.rearrange("(xt p) e -> p xt e", p=P)
    woutT_v = woutT_f16.rearrange("(kt p) d -> p kt d", p=P)

    with TileContext(nc) as tc:
        with (
            tc.tile_pool(name="res", bufs=1) as res,
            tc.tile_pool(name="ps_tr", bufs=2, space="PSUM") as ps_tr_pool,
            tc.tile_pool(name="ps_mm", bufs=2, space="PSUM") as ps_mm_pool,
            tc.tile_pool(name="ps_m34", bufs=2, space="PSUM") as ps_m34_pool,
        ):
            # ---------- resident tensors ----------
            ci_hi = res.tile([P, KD, X], BF16, tag="ci_hi")
            ci_lo = res.tile([P, KD, X], BF16, tag="ci_lo")
            ctx_t = res.tile([P, KX, E2], F16, tag="ctx_t")
            nc.sync.dma_start(out=ctx_t[:], in_=ctx_f16_v[:])
            wout_t = res.tile([P, F // P, D], F16, tag="wout_t")
            nc.sync.dma_start(out=wout_t[:], in_=woutT_v[:])
            ident = res.tile([P, P], F16, tag="ident")
            make_identity(nc, ident[:])
            ones_t = res.tile([1, P], F16, tag="ones_t")
            nc.vector.memset(ones_t[:], 1.0)
            bout_t = res.tile([1, D], F16, tag="bout_t")
            nc.sync.dma_start(out=bout_t[:], in_=bout_f16[:])

            # ---------- phase 1: ci = W_in @ context^T (split-3) ----------
            # per-k-tile tiles so the first matmul only waits on ~1MB of DMA
            with (
                tc.tile_pool(name="ph1w", bufs=1) as ph1w,
                tc.tile_pool(name="ph1", bufs=2) as ph1,
            ):
                win_hi_t, win_lo_t = [], []
                for ke in range(KE):
                    wh = ph1w.tile([P, D], BF16, tag=f"win_hi{ke}")
                    nc.sync.dma_start(out=wh[:], in_=winT_hi_v[:, ke, :])
                    win_hi_t.append(wh)
                    wl = ph1w.tile([P, D], BF16, tag=f"win_lo{ke}")
                    nc.sync.dma_start(out=wl[:], in_=winT_lo_v[:, ke, :])
                    win_lo_t.append(wl)

                for nx in range(NXS):
                    xsl = slice(nx * XS, (nx + 1) * XS)
                    cxh, cxl = [], []
                    for ke in range(KE):
                        ch = ph1.tile([P, XS], BF16, tag=f"cxh{ke}")
                        nc.sync.dma_start(out=ch[:], in_=ctxT_hi_v[:, ke, xsl])
                        cxh.append(ch)
                        cl = ph1.tile([P, XS], BF16, tag=f"cxl{ke}")
                        nc.sync.dma_start(out=cl[:], in_=ctxT_lo_v[:, ke, xsl])
                        cxl.append(cl)
                    for md in range(KD):
                        ps = ps_mm_pool.tile([P, XS], F32, tag="ps_logit")
                        for ke in range(KE):
                            wh = win_hi_t[ke][:, md * P:(md + 1) * P]
                            wl = win_lo_t[ke][:, md * P:(md + 1) * P]
                            nc.tensor.matmul(
                                ps[:], wh, cxh[ke][:],
                                start=(ke == 0), stop=False,
                            )
                            nc.tensor.matmul(ps[:], wh, cxl[ke][:],
                                             start=False, stop=False)
                            nc.tensor.matmul(ps[:], wl, cxh[ke][:],
                                             start=False, stop=(ke == KE - 1))
                        nc.vector.tensor_copy(ci_hi[:, md, xsl], ps[:])
                        nc.vector.scalar_tensor_tensor(
                            out=ci_lo[:, md, xsl], in0=ps[:], scalar=1.0,
                            in1=ci_hi[:, md, xsl],
                            op0=AluOp.mult, op1=AluOp.subtract,
                        )

            # ---------- phase 2: per-y-tile pipeline ----------
            with (
                tc.tile_pool(name="stream", bufs=2) as stream,
                tc.tile_pool(name="stream1", bufs=1) as stream1,
                tc.tile_pool(name="stage", bufs=1) as stage,
                tc.tile_pool(name="e_pool", bufs=2) as e_pool,
            ):
                # per-macro tiles (bufs=1: PE program order serializes reuse)
                attenT = stage.tile([P, KX, Y_MACRO], F16, tag="attenT")
                ctxTout = stage.tile([P, ME, Y_MACRO], F16, tag="ctxTout")

                e_tiles = {}  # ytg -> e_f16 tile (lagged transpose reads)

                def emit_m2_softmax(ytg):
                    ysl = slice(ytg * P, (ytg + 1) * P)
                    oth = stream.tile([P, KD, P], BF16, tag="oth")
                    nc.sync.dma_start(out=oth[:], in_=outT_hi_v[:, :, ysl])
                    otl = stream.tile([P, KD, P], BF16, tag="otl")
                    nc.sync.dma_start(out=otl[:], in_=outT_lo_v[:, :, ysl])
                    keep_t = stream1.tile([P, X], F16, tag="keep_t")
                    nc.sync.dma_start(out=keep_t[:], in_=keep_f16[ysl, :])

                    L = stage.tile([P, X], F32, tag="L")
                    for nx in range(NXS):
                        ps = ps_mm_pool.tile([P, XS], F32, tag="ps_logit")
                        xsl = slice(nx * XS, (nx + 1) * XS)
                        for kd in range(KD):
                            lh = oth[:, kd, :]
                            ll = otl[:, kd, :]
                            nc.tensor.matmul(
                                ps[:], lh, ci_hi[:, kd, xsl],
                                start=(kd == 0), stop=False,
                            )
                            nc.tensor.matmul(ps[:], lh, ci_lo[:, kd, xsl],
                                             start=False, stop=False)
                            nc.tensor.matmul(ps[:], ll, ci_hi[:, kd, xsl],
                                             start=False, stop=(kd == KD - 1))
                        nc.vector.scalar_tensor_tensor(
                            out=L[:, xsl], in0=ps[:], scalar=1.0,
                            in1=keep_t[:, xsl],
                            op0=AluOp.mult, op1=AluOp.mult,
                        )

                    negmax = stage.tile([P, 1], F32, tag="negmax")
                    nc.vector.tensor_reduce(
                        out=negmax[:], in_=L[:], axis=mybir.AxisListType.X,
                        op=AluOp.max, negate=True,
                    )
                    e_t = stage.tile([P, X], F16, tag="e_t")
                    sums = stage.tile([P, 1], F32, tag="sums")
                    nc.scalar.activation(
                        out=e_t[:], in_=L[:], func=ActFn.Exp,
                        bias=negmax[:], scale=1.0, accum_out=sums[:],
                    )
                    recip = stage.tile([P, 1], F32, tag="recip")
                    nc.vector.reciprocal(recip[:], sums[:])
                    att_f16 = e_pool.tile([P, X], F16, tag="att_f16")
                    nc.vector.tensor_scalar_mul(att_f16[:], e_t[:], recip[:])
                    att_sb = stage.tile([P, X], F32, tag="att_sb")
                    nc.scalar.copy(att_sb[:], att_f16[:])
                    nc.sync.dma_start(out=atten_out[ysl, :], in_=att_sb[:])
                    e_tiles[ytg] = att_f16

                def emit_transposes(ytg):
                    e_t = e_tiles.pop(ytg)
                    yt = ytg % 4
                    for xt in range(KX):
                        ps_t = ps_tr_pool.tile([P, P], F16, tag="ps_t")
                        nc.tensor.transpose(
                            ps_t[:], e_t[:, xt * P:(xt + 1) * P], ident[:]
                        )
                        nc.scalar.copy(
                            attenT[:, xt, yt * P:(yt + 1) * P], ps_t[:]
                        )

                def emit_m3_m4(my):
                    # M3: ctxT[e', y_macro] = sum_x context[x, e'] * attenT[x, y]
                    for me in range(ME):
                        ps = ps_m34_pool.tile([P, Y_MACRO], F32, tag="ps_m34")
                        for kx in range(KX):
                            nc.tensor.matmul(
                                ps[:],
                                ctx_t[:, kx, me * P:(me + 1) * P],
                                attenT[:, kx, :],
                                start=(kx == 0), stop=(kx == KX - 1),
                            )
                        nc.scalar.copy(ctxTout[:, me, :], ps[:])

                    # M4 per y-tile of this macro
                    ot16 = stream1.tile([P, KD, Y_MACRO], F16, tag="ot16")
                    nc.sync.dma_start(
                        out=ot16[:],
                        in_=outT_f16_v[:, :, my * Y_MACRO:(my + 1) * Y_MACRO],
                    )
                    for yt in range(4):
                        ytg = my * 4 + yt
                        ysl_g = slice(ytg * P, (ytg + 1) * P)
                        ysl_m = slice(yt * P, (yt + 1) * P)
                        out_sb = stage.tile([P, D], F32, tag="out_sb")
                        for nd in range(ND):
                            ps = ps_m34_pool.tile([P, XS], F32, tag="ps_m34")
                            dsl = slice(nd * XS, (nd + 1) * XS)
                            for k in range(F // P):
                                if k < ME:
                                    lhsT = ctxTout[:, k, ysl_m]
                                else:
                                    lhsT = ot16[:, k - ME, ysl_m]
                                nc.tensor.matmul(
                                    ps[:], lhsT, wout_t[:, k, dsl],
                                    start=(k == 0), stop=False,
                                )
                            nc.tensor.matmul(
                                ps[:], ones_t[:, 0:P], bout_t[:, dsl],
                                start=False, stop=True,
                            )
                            nc.scalar.activation(
                                out=out_sb[:, dsl], in_=ps[:], func=ActFn.Tanh,
                            )
                        nc.sync.dma_start(out=out_out[ysl_g, :], in_=out_sb[:])

                # software-pipelined emission: transposes lag M2 by one tile
                for ytg in range(YT + 1):
                    if ytg < YT:
                        emit_m2_softmax(ytg)
                    if ytg >= 1:
                        emit_transposes(ytg - 1)
                        if (ytg - 1) % 4 == 3:
                            emit_m3_m4((ytg - 1) // 4)

    nc.compile()
    return nc


def _bf16_split(x32):
    hi = x32.astype(ml_dtypes.bfloat16)
    lo = (x32 - hi.astype(np.float32)).astype(ml_dtypes.bfloat16)
    return hi, lo


def host_prep(output, context, masks, W_in, W_out, b_out):
    """Build per-core input maps from the full inputs."""
    output = np.asarray(output, dtype=np.float32)
    context = np.asarray(context, dtype=np.float32)
    masks = np.asarray(masks)
    W_in = np.asarray(W_in, dtype=np.float32)
    W_out = np.asarray(W_out, dtype=np.float32)
    b_out = np.asarray(b_out, dtype=np.float32)

    winT = np.ascontiguousarray(W_in.T)           # [E2, D]
    winT_hi, winT_lo = _bf16_split(winT)
    woutT_f16 = np.ascontiguousarray(W_out.T).astype(np.float16)  # [F, D]
    bout_f16 = b_out.astype(np.float16).reshape(1, D)

    in_maps = []
    for b in range(B):
        ctxT = np.ascontiguousarray(context[b].T)  # [E2, X]
        ctxT_hi, ctxT_lo = _bf16_split(ctxT)
        outT = np.ascontiguousarray(output[b].T)   # [D, Y]
        outT_hi, outT_lo = _bf16_split(outT)
        in_maps.append({
            "winT_hi": winT_hi, "winT_lo": winT_lo,
            "ctxT_hi": ctxT_hi, "ctxT_lo": ctxT_lo,
            "outT_hi": outT_hi, "outT_lo": outT_lo,
            "outT_f16": outT.astype(np.float16),
            "ctx_f16": context[b].astype(np.float16),
            "keep_f16": (1 - masks[b]).astype(np.float16),
            "woutT_f16": woutT_f16,
            "bout_f16": bout_f16,
        })
    return in_maps


_NC_CACHE = {}


def _get_nc():
    if "nc" not in _NC_CACHE:
        _NC_CACHE["nc"] = build_nc()
    return _NC_CACHE["nc"]


def kernel(output, context, masks, W_in, W_out, b_out, _trace=False, _tmpdir=None):
    nc = _get_nc()
    in_maps = host_prep(output, context, masks, W_in, W_out, b_out)
    res = run_bass_kernel_spmd(
        nc, in_maps, list(range(N_CORES)), trace=_trace, tmpdir=_tmpdir
    )
    atten = np.stack([res.results[i]["atten"] for i in range(N_CORES)])
    out = np.stack([res.results[i]["out"] for i in range(N_CORES)])
    kernel.last_exec_time_ns = res.exec_time_ns
    kernel.last_trace = (
        res.instructions_and_trace[1] if res.instructions_and_trace else None
    )
    return (out, atten)


kernel.last_exec_time_ns = None
kernel.last_trace = None


# revision 14
# speedup vs baseline: 1.0354x; 1.0263x over previous
"""Trainium2 Bass kernel for nn_Attention_26920855011759.

Computes, per batch b (sharded 1 batch per NeuronCore, 8 cores):
    ci     = W_in @ context[b]^T                  [D, X]
    logits = output[b] @ ci                       [Y, X]
    logits = where(mask, -1e-6, logits)           (-1e-6 folded to 0.0: logits
                                                   are continuous ~N(0,32), the
                                                   1e-6 shift is ~1e-38 relative
                                                   after softmax)
    atten  = softmax(logits, axis=-1)             [Y, X]  (fp32 output)
    ctx    = atten @ context[b]                   [Y, E2]
    out    = tanh([ctx, output[b]] @ W_out^T + b) [Y, D]  (fp32 output)

Precision: the logit path (ci and logits matmuls) runs as 3-pass bf16
hi/lo split products (hi*hi + hi*lo + lo*hi, fp32 PSUM accumulation),
giving ~fp32-quality logits; softmax runs in fp32 on-chip; the ctx/out
matmuls run in fp16. Expected absmax error ~2e-4 (atten) / ~2e-3 (out).
"""

import sys

if "/opt/trn_rl_repo" not in sys.path:
    sys.path.insert(0, "/opt/trn_rl_repo")

import numpy as np
import ml_dtypes

import concourse.bass as bass
import concourse.bacc as bacc
import concourse.mybir as mybir
from concourse.tile import TileContext
from concourse.bass_utils import run_bass_kernel_spmd
from concourse.masks import make_identity

F32 = mybir.dt.float32
BF16 = mybir.dt.bfloat16
F16 = mybir.dt.float16

B, Y, X = 8, 2048, 2048
E2 = 1024  # encoder_hidden_size * 2 ("e" axis)
D = 1024   # decoder_hidden_size ("d" axis)
F = E2 + D # concat axis for the output projection
P = 128    # partitions
N_CORES = 8

YT = Y // P          # 16 y tiles
Y_MACRO = 512        # y macro-tile (M3/M4 free dim)
YM = Y // Y_MACRO    # 4 y macros
XS = 512             # x slice (M2 free dim / PSUM bank)
NXS = X // XS        # 4 x slices
KE = E2 // P         # 8 contraction tiles over e
KD = D // P          # 8 contraction tiles over d
KX = X // P          # 16 contraction tiles over x
ME = E2 // P         # 8 output-partition tiles over e (M3)
ND = D // XS         # 2 d slices (M4 free dim)

AluOp = mybir.AluOpType
ActFn = mybir.ActivationFunctionType


def build_nc():
    nc = bacc.Bacc()

    # --- DRAM parameters (per-core shard; names match host prep below) ---
    winT_hi = nc.declare_dram_parameter("winT_hi", [E2, D], BF16, isOutput=False)
    winT_lo = nc.declare_dram_parameter("winT_lo", [E2, D], BF16, isOutput=False)
    ctxT_hi = nc.declare_dram_parameter("ctxT_hi", [E2, X], BF16, isOutput=False)
    ctxT_lo = nc.declare_dram_parameter("ctxT_lo", [E2, X], BF16, isOutput=False)
    outT_hi = nc.declare_dram_parameter("outT_hi", [D, Y], BF16, isOutput=False)
    outT_lo = nc.declare_dram_parameter("outT_lo", [D, Y], BF16, isOutput=False)
    outT_f16 = nc.declare_dram_parameter("outT_f16", [D, Y], F16, isOutput=False)
    ctx_f16 = nc.declare_dram_parameter("ctx_f16", [X, E2], F16, isOutput=False)
    keep_f16 = nc.declare_dram_parameter("keep_f16", [Y, X], F16, isOutput=False)
    woutT_f16 = nc.declare_dram_parameter("woutT_f16", [F, D], F16, isOutput=False)
    bout_f16 = nc.declare_dram_parameter("bout_f16", [1, D], F16, isOutput=False)

    atten_out = nc.declare_dram_parameter("atten", [Y, X], F32, isOutput=True)
    out_out = nc.declare_dram_parameter("out", [Y, D], F32, isOutput=True)

    # tiled dram views: partition-major [p, tile, free]
    winT_hi_v = winT_hi.rearrange("(kt p) d -> p kt d", p=P)
    winT_lo_v = winT_lo.rearrange("(kt p) d -> p kt d", p=P)
    ctxT_hi_v = ctxT_hi.rearrange("(kt p) x -> p kt x", p=P)
    ctxT_lo_v = ctxT_lo.rearrange("(kt p) x -> p kt x", p=P)
    outT_hi_v = outT_hi.rearrange("(kt p) y -> p kt y", p=P)
    outT_lo_v = outT_lo.rearrange("(kt p) y -> p kt y", p=P)
    outT_f16_v = outT_f16.rearrange("(kt p) y -> p kt y", p=P)
    ctx_f16_v = ctx_f16.rearrange("(xt p) e -> p xt e", p=P)
    woutT_v = woutT_f16.rearrange("(kt p) d -> p kt d", p=P)

    with TileContext(nc) as tc:
        with tc.tile_pool(name="res", bufs=1) as res:
            # ---------- resident tensors ----------
            ci_hi = res.tile([P, KD, X], BF16, tag="ci_hi")
            ci_lo = res.tile([P, KD, X], BF16, tag="ci_lo")
            ctx_t = res.tile([P, KX, E2], F16, tag="ctx_t")
            wout_t = res.tile([P, F // P, D], F16, tag="wout_t")
            ident = res.tile([P, P], F16, tag="ident")
            make_identity(nc, ident[:])
            ones_t = res.tile([1, P], F16, tag="ones_t")
            nc.vector.memset(ones_t[:], 1.0)
            bout_t = res.tile([1, D], F16, tag="bout_t")
            nc.sync.dma_start(out=bout_t[:], in_=bout_f16[:])

            # ---------- phase 1: ci = W_in @ context^T (split-3) ----------
            # per-k-tile tiles so the first matmul only waits on ~1MB of DMA
            with (
                tc.tile_pool(name="ph1w", bufs=1) as ph1w,
                tc.tile_pool(name="ph1", bufs=2) as ph1,
                tc.tile_pool(name="ph1_ps", bufs=1, space="PSUM") as ph1_ps,
            ):
                win_hi_t, win_lo_t = [], []
                for ke in range(KE):
                    wh = ph1w.tile([P, D], BF16, tag=f"win_hi{ke}")
                    nc.sync.dma_start(out=wh[:], in_=winT_hi_v[:, ke, :])
                    win_hi_t.append(wh)
                    wl = ph1w.tile([P, D], BF16, tag=f"win_lo{ke}")
                    nc.sync.dma_start(out=wl[:], in_=winT_lo_v[:, ke, :])
                    win_lo_t.append(wl)

                for nx in range(NXS):
                    xsl = slice(nx * XS, (nx + 1) * XS)
                    cxh, cxl = [], []
                    for ke in range(KE):
                        ch = ph1.tile([P, XS], BF16, tag=f"cxh{ke}")
                        nc.sync.dma_start(out=ch[:], in_=ctxT_hi_v[:, ke, xsl])
                        cxh.append(ch)
                        cl = ph1.tile([P, XS], BF16, tag=f"cxl{ke}")
                        nc.sync.dma_start(out=cl[:], in_=ctxT_lo_v[:, ke, xsl])
                        cxl.append(cl)
                    for mg in range(2):  # md groups of 4: bounded DMA demand
                        pss = [
                            ph1_ps.tile([P, XS], F32, tag=f"ps_ci{j}",
                                        name=f"ps_ci{nx}_{mg}_{j}")
                            for j in range(4)
                        ]
                        for ke in range(KE):
                            for j in range(4):
                                md = mg * 4 + j
                                wh = win_hi_t[ke][:, md * P:(md + 1) * P]
                                wl = win_lo_t[ke][:, md * P:(md + 1) * P]
                                nc.tensor.matmul(
                                    pss[j][:], wh, cxh[ke][:],
                                    start=(ke == 0), stop=False,
                                )
                                nc.tensor.matmul(pss[j][:], wh, cxl[ke][:],
                                                 start=False, stop=False)
                                nc.tensor.matmul(pss[j][:], wl, cxh[ke][:],
                                                 start=False,
                                                 stop=(ke == KE - 1))
                        for j in range(4):
                            md = mg * 4 + j
                            nc.vector.tensor_copy(ci_hi[:, md, xsl], pss[j][:])
                            nc.vector.scalar_tensor_tensor(
                                out=ci_lo[:, md, xsl], in0=pss[j][:], scalar=1.0,
                                in1=ci_hi[:, md, xsl],
                                op0=AluOp.mult, op1=AluOp.subtract,
                            )

            # ---------- phase 2: per-y-tile pipeline ----------
            with (
                tc.tile_pool(name="stream", bufs=2) as stream,
                tc.tile_pool(name="stream1", bufs=1) as stream1,
                tc.tile_pool(name="stage", bufs=1) as stage,
                tc.tile_pool(name="e_pool", bufs=2) as e_pool,
                tc.tile_pool(name="ps_tr", bufs=2, space="PSUM") as ps_tr_pool,
                tc.tile_pool(name="ps_mm", bufs=2, space="PSUM") as ps_mm_pool,
                tc.tile_pool(name="ps_m34", bufs=2, space="PSUM") as ps_m34_pool,
            ):
                nc.sync.dma_start(out=ctx_t[:], in_=ctx_f16_v[:])
                nc.sync.dma_start(out=wout_t[:], in_=woutT_v[:])
                # per-macro tiles (bufs=1: PE program order serializes reuse)
                attenT = stage.tile([P, KX, Y_MACRO], F16, tag="attenT")
                ctxTout = stage.tile([P, ME, Y_MACRO], F16, tag="ctxTout")

                e_tiles = {}  # ytg -> e_f16 tile (lagged transpose reads)

                def emit_m2_softmax(ytg):
                    ysl = slice(ytg * P, (ytg + 1) * P)
                    oth = stream.tile([P, KD, P], BF16, tag="oth")
                    nc.sync.dma_start(out=oth[:], in_=outT_hi_v[:, :, ysl])
                    otl = stream.tile([P, KD, P], BF16, tag="otl")
                    nc.sync.dma_start(out=otl[:], in_=outT_lo_v[:, :, ysl])
                    keep_t = stream1.tile([P, X], F16, tag="keep_t")
                    nc.sync.dma_start(out=keep_t[:], in_=keep_f16[ysl, :])

                    L = stage.tile([P, X], F32, tag="L")
                    for nx in range(NXS):
                        ps = ps_mm_pool.tile([P, XS], F32, tag="ps_logit")
                        xsl = slice(nx * XS, (nx + 1) * XS)
                        for kd in range(KD):
                            lh = oth[:, kd, :]
                            ll = otl[:, kd, :]
                            nc.tensor.matmul(
                                ps[:], lh, ci_hi[:, kd, xsl],
                                start=(kd == 0), stop=False,
                            )
                            nc.tensor.matmul(ps[:], lh, ci_lo[:, kd, xsl],
                                             start=False, stop=False)
                            nc.tensor.matmul(ps[:], ll, ci_hi[:, kd, xsl],
                                             start=False, stop=(kd == KD - 1))
                        nc.vector.scalar_tensor_tensor(
                            out=L[:, xsl], in0=ps[:], scalar=1.0,
                            in1=keep_t[:, xsl],
                            op0=AluOp.mult, op1=AluOp.mult,
                        )

                    negmax = stage.tile([P, 1], F32, tag="negmax")
                    nc.vector.tensor_reduce(
                        out=negmax[:], in_=L[:], axis=mybir.AxisListType.X,
                        op=AluOp.max, negate=True,
                    )
                    e_t = stage.tile([P, X], F16, tag="e_t")
                    sums = stage.tile([P, 1], F32, tag="sums")
                    nc.scalar.activation(
                        out=e_t[:], in_=L[:], func=ActFn.Exp,
                        bias=negmax[:], scale=1.0, accum_out=sums[:],
                    )
                    recip = stage.tile([P, 1], F32, tag="recip")
                    nc.vector.reciprocal(recip[:], sums[:])
                    att_f16 = e_pool.tile([P, X], F16, tag="att_f16")
                    nc.vector.tensor_scalar_mul(att_f16[:], e_t[:], recip[:])
                    att_sb = stage.tile([P, X], F32, tag="att_sb")
                    nc.scalar.copy(att_sb[:], att_f16[:])
                    nc.sync.dma_start(out=atten_out[ysl, :], in_=att_sb[:])
                    e_tiles[ytg] = att_f16

                def emit_transposes(ytg):
                    e_t = e_tiles.pop(ytg)
                    yt = ytg % 4
                    for xt in range(KX):
                        ps_t = ps_tr_pool.tile([P, P], F16, tag="ps_t")
                        nc.tensor.transpose(
                            ps_t[:], e_t[:, xt * P:(xt + 1) * P], ident[:]
                        )
                        nc.scalar.copy(
                            attenT[:, xt, yt * P:(yt + 1) * P], ps_t[:]
                        )

                def emit_m3_m4(my):
                    # M3: ctxT[e', y_macro] = sum_x context[x, e'] * attenT[x, y]
                    for me in range(ME):
                        ps = ps_m34_pool.tile([P, Y_MACRO], F32, tag="ps_m34")
                        for kx in range(KX):
                            nc.tensor.matmul(
                                ps[:],
                                ctx_t[:, kx, me * P:(me + 1) * P],
                                attenT[:, kx, :],
                                start=(kx == 0), stop=(kx == KX - 1),
                            )
                        nc.scalar.copy(ctxTout[:, me, :], ps[:])

                    # M4 per y-tile of this macro
                    ot16 = stream1.tile([P, KD, Y_MACRO], F16, tag="ot16")
                    nc.sync.dma_start(
                        out=ot16[:],
                        in_=outT_f16_v[:, :, my * Y_MACRO:(my + 1) * Y_MACRO],
                    )
                    for yt in range(4):
                        ytg = my * 4 + yt
                        ysl_g = slice(ytg * P, (ytg + 1) * P)
                        ysl_m = slice(yt * P, (yt + 1) * P)
                        out_sb = stage.tile([P, D], F32, tag="out_sb")
                        for nd in range(ND):
                            ps = ps_m34_pool.tile([P, XS], F32, tag="ps_m34")
                            dsl = slice(nd * XS, (nd + 1) * XS)
                            for k in range(F // P):
                                if k < ME:
                                    lhsT = ctxTout[:, k, ysl_m]
                                else:
                                    lhsT = ot16[:, k - ME, ysl_m]
                                nc.tensor.matmul(
                                    ps[:], lhsT, wout_t[:, k, dsl],
                                    start=(k == 0), stop=False,
                                )
                            nc.tensor.matmul(
                                ps[:], ones_t[:, 0:P], bout_t[:, dsl],
                                start=False, stop=True,
                            )
                            nc.scalar.activation(
                                out=out_sb[:, dsl], in_=ps[:], func=ActFn.Tanh,
                            )
                        nc.sync.dma_start(out=out_out[ysl_g, :], in_=out_sb[:])

                # software-pipelined emission: transposes lag M2 by one tile
                for ytg in range(YT + 1):
                    if ytg < YT:
                        emit_m2_softmax(ytg)
                    if ytg >= 1:
                        emit_transposes(ytg - 1)
                        if (ytg - 1) % 4 == 3:
                            emit_m3_m4((ytg - 1) // 4)

    nc.compile()
    return nc


def _bf16_split(x32):
    hi = x32.astype(ml_dtypes.bfloat16)
    lo = (x32 - hi.astype(np.float32)).astype(ml_dtypes.bfloat16)
    return hi, lo


def host_prep(output, context, masks, W_in, W_out, b_out):
    """Build per-core input maps from the full inputs."""
    output = np.asarray(output, dtype=np.float32)
    context = np.asarray(context, dtype=np.float32)
    masks = np.asarray(masks)
    W_in = np.asarray(W_in, dtype=np.float32)
    W_out = np.asarray(W_out, dtype=np.float32)
    b_out = np.asarray(b_out, dtype=np.float32)

    winT = np.ascontiguousarray(W_in.T)           # [E2, D]
    winT_hi, winT_lo = _bf16_split(winT)
    woutT_f16 = np.ascontiguousarray(W_out.T).astype(np.float16)  # [F, D]
    bout_f16 = b_out.astype(np.float16).reshape(1, D)

    in_maps = []
    for b in range(B):
        ctxT = np.ascontiguousarray(context[b].T)  # [E2, X]
        ctxT_hi, ctxT_lo = _bf16_split(ctxT)
        outT = np.ascontiguousarray(output[b].T)   # [D, Y]
        outT_hi, outT_lo = _bf16_split(outT)
        in_maps.append({
            "winT_hi": winT_hi, "winT_lo": winT_lo,
            "ctxT_hi": ctxT_hi, "ctxT_lo": ctxT_lo,
            "outT_hi": outT_hi, "outT_lo": outT_lo,
            "outT_f16": outT.astype(np.float16),
            "ctx_f16": context[b].astype(np.float16),
            "keep_f16": (1 - masks[b]).astype(np.float16),
            "woutT_f16": woutT_f16,
            "bout_f16": bout_f16,
        })
    return in_maps


_NC_CACHE = {}


def _get_nc():
    if "nc" not in _NC_CACHE:
        _NC_CACHE["nc"] = build_nc()
    return _NC_CACHE["nc"]


def kernel(output, context, masks, W_in, W_out, b_out, _trace=False, _tmpdir=None):
    nc = _get_nc()
    in_maps = host_prep(output, context, masks, W_in, W_out, b_out)
    res = run_bass_kernel_spmd(
        nc, in_maps, list(range(N_CORES)), trace=_trace, tmpdir=_tmpdir
    )
    atten = np.stack([res.results[i]["atten"] for i in range(N_CORES)])
    out = np.stack([res.results[i]["out"] for i in range(N_CORES)])
    kernel.last_exec_time_ns = res.exec_time_ns
    kernel.last_trace = (
        res.instructions_and_trace[1] if res.instructions_and_trace else None
    )
    return (out, atten)


kernel.last_exec_time_ns = None
kernel.last_trace = None


# revision 15
# speedup vs baseline: 1.0484x; 1.0126x over previous
"""Trainium2 Bass kernel for nn_Attention_26920855011759.

Computes, per batch b (sharded 1 batch per NeuronCore, 8 cores):
    ci     = W_in @ context[b]^T                  [D, X]
    logits = output[b] @ ci                       [Y, X]
    logits = where(mask, -1e-6, logits)           (-1e-6 folded to 0.0: logits
                                                   are continuous ~N(0,32), the
                                                   1e-6 shift is ~1e-38 relative
                                                   after softmax)
    atten  = softmax(logits, axis=-1)             [Y, X]  (fp32 output)
    ctx    = atten @ context[b]                   [Y, E2]
    out    = tanh([ctx, output[b]] @ W_out^T + b) [Y, D]  (fp32 output)

Precision: the logit path (ci and logits matmuls) runs as 3-pass bf16
hi/lo split products (hi*hi + hi*lo + lo*hi, fp32 PSUM accumulation),
giving ~fp32-quality logits; softmax runs in fp32 on-chip; the ctx/out
matmuls run in fp16. Expected absmax error ~2e-4 (atten) / ~2e-3 (out).
"""

import sys

if "/opt/trn_rl_repo" not in sys.path:
    sys.path.insert(0, "/opt/trn_rl_repo")

import numpy as np
import ml_dtypes

import concourse.bass as bass
import concourse.bacc as bacc
import concourse.mybir as mybir
from concourse.tile import TileContext
from concourse.bass_utils import run_bass_kernel_spmd
from concourse.masks import make_identity

F32 = mybir.dt.float32
BF16 = mybir.dt.bfloat16
F16 = mybir.dt.float16

B, Y, X = 8, 2048, 2048
E2 = 1024  # encoder_hidden_size * 2 ("e" axis)
D = 1024   # decoder_hidden_size ("d" axis)
F = E2 + D # concat axis for the output projection
P = 128    # partitions
N_CORES = 8

YT = Y // P          # 16 y tiles
Y_MACRO = 512        # y macro-tile (M3/M4 free dim)
YM = Y // Y_MACRO    # 4 y macros
XS = 512             # x slice (M2 free dim / PSUM bank)
NXS = X // XS        # 4 x slices
KE = E2 // P         # 8 contraction tiles over e
KD = D // P          # 8 contraction tiles over d
KX = X // P          # 16 contraction tiles over x
ME = E2 // P         # 8 output-partition tiles over e (M3)
ND = D // XS         # 2 d slices (M4 free dim)

AluOp = mybir.AluOpType
ActFn = mybir.ActivationFunctionType


def build_nc():
    nc = bacc.Bacc()

    # --- DRAM parameters (per-core shard; names match host prep below) ---
    winT_hi = nc.declare_dram_parameter("winT_hi", [E2, D], BF16, isOutput=False)
    winT_lo = nc.declare_dram_parameter("winT_lo", [E2, D], BF16, isOutput=False)
    ctxT_hi = nc.declare_dram_parameter("ctxT_hi", [E2, X], BF16, isOutput=False)
    ctxT_lo = nc.declare_dram_parameter("ctxT_lo", [E2, X], BF16, isOutput=False)
    outT_hi = nc.declare_dram_parameter("outT_hi", [D, Y], BF16, isOutput=False)
    outT_lo = nc.declare_dram_parameter("outT_lo", [D, Y], BF16, isOutput=False)
    outT_f16 = nc.declare_dram_parameter("outT_f16", [D, Y], F16, isOutput=False)
    ctx_f16 = nc.declare_dram_parameter("ctx_f16", [X, E2], F16, isOutput=False)
    keep_f16 = nc.declare_dram_parameter("keep_f16", [Y, X], F16, isOutput=False)
    woutT_f16 = nc.declare_dram_parameter("woutT_f16", [F, D], F16, isOutput=False)
    bout_f16 = nc.declare_dram_parameter("bout_f16", [1, D], F16, isOutput=False)

    atten_out = nc.declare_dram_parameter("atten", [Y, X], F32, isOutput=True)
    out_out = nc.declare_dram_parameter("out", [Y, D], F32, isOutput=True)

    # tiled dram views: partition-major [p, tile, free]
    winT_hi_v = winT_hi.rearrange("(kt p) d -> p kt d", p=P)
    winT_lo_v = winT_lo.rearrange("(kt p) d -> p kt d", p=P)
    ctxT_hi_v = ctxT_hi.rearrange("(kt p) x -> p kt x", p=P)
    ctxT_lo_v = ctxT_lo.rearrange("(kt p) x -> p kt x", p=P)
    outT_hi_v = outT_hi.rearrange("(kt p) y -> p kt y", p=P)
    outT_lo_v = outT_lo.rearrange("(kt p) y -> p kt y", p=P)
    outT_f16_v = outT_f16.rearrange("(kt p) y -> p kt y", p=P)
    ctx_f16_v = ctx_f16.rearrange("(xt p) e -> p xt e", p=P)
    woutT_v = woutT_f16.rearrange("(kt p) d -> p kt d", p=P)

    with TileContext(nc) as tc:
        with tc.tile_pool(name="res", bufs=1) as res:
            # ---------- resident tensors ----------
            ci_hi = res.tile([P, KD, X], BF16, tag="ci_hi")
            ci_lo = res.tile([P, KD, X], BF16, tag="ci_lo")
            ctx_t = res.tile([P, KX, E2], F16, tag="ctx_t")
            wout_t = res.tile([P, F // P, D], F16, tag="wout_t")
            ident = res.tile([P, P], F16, tag="ident")
            make_identity(nc, ident[:])
            ones_t = res.tile([1, P], F16, tag="ones_t")
            nc.vector.memset(ones_t[:], 1.0)
            bout_t = res.tile([1, D], F16, tag="bout_t")
            nc.sync.dma_start(out=bout_t[:], in_=bout_f16[:])

            # ---------- phase 1: ci = W_in @ context^T (split-3) ----------
            # per-k-tile tiles so the first matmul only waits on ~1MB of DMA
            with (
                tc.tile_pool(name="ph1w", bufs=1) as ph1w,
                tc.tile_pool(name="ph1", bufs=2) as ph1,
                tc.tile_pool(name="ph1_ps", bufs=1, space="PSUM") as ph1_ps,
            ):
                win_hi_t, win_lo_t = [], []
                for ke in range(KE):
                    wh = ph1w.tile([P, D], BF16, tag=f"win_hi{ke}")
                    win_hi_t.append(wh)
                    wl = ph1w.tile([P, D], BF16, tag=f"win_lo{ke}")
                    win_lo_t.append(wl)

                for nx in range(NXS):
                    xsl = slice(nx * XS, (nx + 1) * XS)
                    cxh, cxl = [], []
                    for ke in range(KE):
                        # interleave weight + context chunk DMAs in the order
                        # the first mg-group consumes them
                        if nx == 0:
                            nc.sync.dma_start(
                                out=win_hi_t[ke][:], in_=winT_hi_v[:, ke, :]
                            )
                        ch = ph1.tile([P, XS], BF16, tag=f"cxh{ke}")
                        nc.sync.dma_start(out=ch[:], in_=ctxT_hi_v[:, ke, xsl])
                        cxh.append(ch)
                        cl = ph1.tile([P, XS], BF16, tag=f"cxl{ke}")
                        nc.sync.dma_start(out=cl[:], in_=ctxT_lo_v[:, ke, xsl])
                        cxl.append(cl)
                        if nx == 0:
                            nc.sync.dma_start(
                                out=win_lo_t[ke][:], in_=winT_lo_v[:, ke, :]
                            )
                    for mg in range(2):  # md groups of 4: bounded DMA demand
                        pss = [
                            ph1_ps.tile([P, XS], F32, tag=f"ps_ci{j}",
                                        name=f"ps_ci{nx}_{mg}_{j}")
                            for j in range(4)
                        ]
                        for ke in range(KE):
                            for j in range(4):
                                md = mg * 4 + j
                                wh = win_hi_t[ke][:, md * P:(md + 1) * P]
                                wl = win_lo_t[ke][:, md * P:(md + 1) * P]
                                nc.tensor.matmul(
                                    pss[j][:], wh, cxh[ke][:],
                                    start=(ke == 0), stop=False,
                                )
                                nc.tensor.matmul(pss[j][:], wh, cxl[ke][:],
                                                 start=False, stop=False)
                                nc.tensor.matmul(pss[j][:], wl, cxh[ke][:],
                                                 start=False,
                                                 stop=(ke == KE - 1))
                        for j in range(4):
                            md = mg * 4 + j
                            nc.vector.tensor_copy(ci_hi[:, md, xsl], pss[j][:])
                            nc.vector.scalar_tensor_tensor(
                                out=ci_lo[:, md, xsl], in0=pss[j][:], scalar=1.0,
                                in1=ci_hi[:, md, xsl],
                                op0=AluOp.mult, op1=AluOp.subtract,
                            )

            # ---------- phase 2: per-y-tile pipeline ----------
            with (
                tc.tile_pool(name="stream", bufs=2) as stream,
                tc.tile_pool(name="stream1", bufs=1) as stream1,
                tc.tile_pool(name="stage", bufs=1) as stage,
                tc.tile_pool(name="e_pool", bufs=2) as e_pool,
                tc.tile_pool(name="ps_tr", bufs=2, space="PSUM") as ps_tr_pool,
                tc.tile_pool(name="ps_mm", bufs=2, space="PSUM") as ps_mm_pool,
                tc.tile_pool(name="ps_m34", bufs=2, space="PSUM") as ps_m34_pool,
            ):
                nc.sync.dma_start(out=ctx_t[:], in_=ctx_f16_v[:])
                nc.sync.dma_start(out=wout_t[:], in_=woutT_v[:])
                # per-macro tiles (bufs=1: PE program order serializes reuse)
                attenT = stage.tile([P, KX, Y_MACRO], F16, tag="attenT")
                ctxTout = stage.tile([P, ME, Y_MACRO], F16, tag="ctxTout")

                e_tiles = {}  # ytg -> e_f16 tile (lagged transpose reads)

                def emit_m2_softmax(ytg):
                    ysl = slice(ytg * P, (ytg + 1) * P)
                    oth = stream.tile([P, KD, P], BF16, tag="oth")
                    nc.sync.dma_start(out=oth[:], in_=outT_hi_v[:, :, ysl])
                    otl = stream.tile([P, KD, P], BF16, tag="otl")
                    nc.sync.dma_start(out=otl[:], in_=outT_lo_v[:, :, ysl])
                    keep_t = stream1.tile([P, X], F16, tag="keep_t")
                    nc.sync.dma_start(out=keep_t[:], in_=keep_f16[ysl, :])

                    L = stage.tile([P, X], F32, tag="L")
                    for nx in range(NXS):
                        ps = ps_mm_pool.tile([P, XS], F32, tag="ps_logit")
                        xsl = slice(nx * XS, (nx + 1) * XS)
                        for kd in range(KD):
                            lh = oth[:, kd, :]
                            ll = otl[:, kd, :]
                            nc.tensor.matmul(
                                ps[:], lh, ci_hi[:, kd, xsl],
                                start=(kd == 0), stop=False,
                            )
                            nc.tensor.matmul(ps[:], lh, ci_lo[:, kd, xsl],
                                             start=False, stop=False)
                            nc.tensor.matmul(ps[:], ll, ci_hi[:, kd, xsl],
                                             start=False, stop=(kd == KD - 1))
                        nc.vector.scalar_tensor_tensor(
                            out=L[:, xsl], in0=ps[:], scalar=1.0,
                            in1=keep_t[:, xsl],
                            op0=AluOp.mult, op1=AluOp.mult,
                        )

                    negmax = stage.tile([P, 1], F32, tag="negmax")
                    nc.vector.tensor_reduce(
                        out=negmax[:], in_=L[:], axis=mybir.AxisListType.X,
                        op=AluOp.max, negate=True,
                    )
                    e_t = stage.tile([P, X], F16, tag="e_t")
                    sums = stage.tile([P, 1], F32, tag="sums")
                    nc.scalar.activation(
                        out=e_t[:], in_=L[:], func=ActFn.Exp,
                        bias=negmax[:], scale=1.0, accum_out=sums[:],
                    )
                    recip = stage.tile([P, 1], F32, tag="recip")
                    nc.vector.reciprocal(recip[:], sums[:])
                    att_f16 = e_pool.tile([P, X], F16, tag="att_f16")
                    nc.vector.tensor_scalar_mul(att_f16[:], e_t[:], recip[:])
                    att_sb = stage.tile([P, X], F32, tag="att_sb")
                    nc.scalar.copy(att_sb[:], att_f16[:])
                    nc.sync.dma_start(out=atten_out[ysl, :], in_=att_sb[:])
                    e_tiles[ytg] = att_f16

                def emit_transposes(ytg):
                    e_t = e_tiles.pop(ytg)
                    yt = ytg % 4
                    for xt in range(KX):
                        ps_t = ps_tr_pool.tile([P, P], F16, tag="ps_t")
                        nc.tensor.transpose(
                            ps_t[:], e_t[:, xt * P:(xt + 1) * P], ident[:]
                        )
                        nc.scalar.copy(
                            attenT[:, xt, yt * P:(yt + 1) * P], ps_t[:]
                        )

                def emit_m3_m4(my):
                    # M3: ctxT[e', y_macro] = sum_x context[x, e'] * attenT[x, y]
                    for me in range(ME):
                        ps = ps_m34_pool.tile([P, Y_MACRO], F32, tag="ps_m34")
                        for kx in range(KX):
                            nc.tensor.matmul(
                                ps[:],
                                ctx_t[:, kx, me * P:(me + 1) * P],
                                attenT[:, kx, :],
                                start=(kx == 0), stop=(kx == KX - 1),
                            )
                        nc.scalar.copy(ctxTout[:, me, :], ps[:])

                    # M4 per y-tile of this macro
                    ot16 = stream1.tile([P, KD, Y_MACRO], F16, tag="ot16")
                    nc.sync.dma_start(
                        out=ot16[:],
                        in_=outT_f16_v[:, :, my * Y_MACRO:(my + 1) * Y_MACRO],
                    )
                    for yt in range(4):
                        ytg = my * 4 + yt
                        ysl_g = slice(ytg * P, (ytg + 1) * P)
                        ysl_m = slice(yt * P, (yt + 1) * P)
                        out_sb = stage.tile([P, D], F32, tag="out_sb")
                        for nd in range(ND):
                            ps = ps_m34_pool.tile([P, XS], F32, tag="ps_m34")
                            dsl = slice(nd * XS, (nd + 1) * XS)
                            for k in range(F // P):
                                if k < ME:
                                    lhsT = ctxTout[:, k, ysl_m]
                                else:
                                    lhsT = ot16[:, k - ME, ysl_m]
                                nc.tensor.matmul(
                                    ps[:], lhsT, wout_t[:, k, dsl],
                                    start=(k == 0), stop=False,
                                )
                            nc.tensor.matmul(
                                ps[:], ones_t[:, 0:P], bout_t[:, dsl],
                                start=False, stop=True,
                            )
                            nc.scalar.activation(
                                out=out_sb[:, dsl], in_=ps[:], func=ActFn.Tanh,
                            )
                        nc.sync.dma_start(out=out_out[ysl_g, :], in_=out_sb[:])

                # software-pipelined emission: transposes lag M2 by one tile
                for ytg in range(YT + 1):
                    if ytg < YT:
                        emit_m2_softmax(ytg)
                    if ytg >= 1:
                        emit_transposes(ytg - 1)
                        if (ytg - 1) % 4 == 3:
                            emit_m3_m4((ytg - 1) // 4)

    nc.compile()
    return nc


def _bf16_split(x32):
    hi = x32.astype(ml_dtypes.bfloat16)
    lo = (x32 - hi.astype(np.float32)).astype(ml_dtypes.bfloat16)
    return hi, lo


def host_prep(output, context, masks, W_in, W_out, b_out):
    """Build per-core input maps from the full inputs."""
    output = np.asarray(output, dtype=np.float32)
    context = np.asarray(context, dtype=np.float32)
    masks = np.asarray(masks)
    W_in = np.asarray(W_in, dtype=np.float32)
    W_out = np.asarray(W_out, dtype=np.float32)
    b_out = np.asarray(b_out, dtype=np.float32)

    winT = np.ascontiguousarray(W_in.T)           # [E2, D]
    winT_hi, winT_lo = _bf16_split(winT)
    woutT_f16 = np.ascontiguousarray(W_out.T).astype(np.float16)  # [F, D]
    bout_f16 = b_out.astype(np.float16).reshape(1, D)

    in_maps = []
    for b in range(B):
        ctxT = np.ascontiguousarray(context[b].T)  # [E2, X]
        ctxT_hi, ctxT_lo = _bf16_split(ctxT)
        outT = np.ascontiguousarray(output[b].T)   # [D, Y]
        outT_hi, outT_lo = _bf16_split(outT)
        in_maps.append({
            "winT_hi": winT_hi, "winT_lo": winT_lo,
            "ctxT_hi": ctxT_hi, "ctxT_lo": ctxT_lo,
            "outT_hi": outT_hi, "outT_lo": outT_lo,
            "outT_f16": outT.astype(np.float16),
            "ctx_f16": context[b].astype(np.float16),
            "keep_f16": (1 - masks[b]).astype(np.float16),
            "woutT_f16": woutT_f16,
            "bout_f16": bout_f16,
        })
    return in_maps


_NC_CACHE = {}


def _get_nc():
    if "nc" not in _NC_CACHE:
        _NC_CACHE["nc"] = build_nc()
    return _NC_CACHE["nc"]


def kernel(output, context, masks, W_in, W_out, b_out, _trace=False, _tmpdir=None):
    nc = _get_nc()
    in_maps = host_prep(output, context, masks, W_in, W_out, b_out)
    res = run_bass_kernel_spmd(
        nc, in_maps, list(range(N_CORES)), trace=_trace, tmpdir=_tmpdir
    )
    atten = np.stack([res.results[i]["atten"] for i in range(N_CORES)])
    out = np.stack([res.results[i]["out"] for i in range(N_CORES)])
    kernel.last_exec_time_ns = res.exec_time_ns
    kernel.last_trace = (
        res.instructions_and_trace[1] if res.instructions_and_trace else None
    )
    return (out, atten)


kernel.last_exec_time_ns = None
kernel.last_trace = None
